# revision 1
# baseline (speedup 1.0000x reference)
"""Sparse L1-distance attention (nn_L1AttnSparse) on 8 Trainium2 NeuronCores.

Sharding: dst tokens are split across the 8 cores (256 dst tokens each);
every core keeps the full k/v tables (8 MB each) in DRAM and uses the
custom SWDGE gather instruction (dma_gather / InstDMAGatherAnt) to pull
the 2 KB k/v rows for its edges.  Scores, softmax over the 32 slots and
the weighted v-sum run on DVE/ACT.  Batch index is folded into the gather
index (tables are [2*2048, 512]).
"""

import sys

sys.path.insert(0, "/opt/trn_rl_repo")

import numpy as np

import concourse.bass as bass
import concourse.tile as tile
from concourse import bacc, mybir
from concourse.bass_utils import run_bass_kernel_spmd

BS = 2
N_TOK = 2048
NH = 8
W = 64
S = 32  # dst_mxlen
HW = NH * W  # 512 floats per (b, tok) row
N_CORES = 8
DT = N_TOK // N_CORES  # dst tokens per core = 256
CHUNKS = DT // 128  # dst chunks of 128 per core = 2
SH = 4  # slot halves per chunk (gather granularity)
SLOTS_PER = S // SH  # 16
IDX_PER = 128 * SLOTS_PER  # 2048 indices per gather


def _wrap_idx(flat):
    """int16 index list -> [128, n/16] tile layout: idx i at [i%16, i//16],
    replicated down the 8 groups of 16 partitions."""
    n = flat.shape[0]
    w16 = np.zeros((16, n // 16), dtype=np.int16)
    w16[np.arange(n) % 16, np.arange(n) // 16] = flat
    return np.tile(w16, (8, 1))


def build_kernel():
    nc = bacc.Bacc(
        "TRN2", target_bir_lowering=False, debug=False, num_devices=N_CORES,
        dynamic_dma_scratch_size=16384 * 8,
    )
    f32 = mybir.dt.float32
    i16 = mybir.dt.int16

    kf = nc.dram_tensor("kf", [BS * N_TOK, HW], f32, kind="ExternalInput").ap()
    vf = nc.dram_tensor("vf", [BS * N_TOK, HW], f32, kind="ExternalInput").ap()
    qc = nc.dram_tensor("qc", [BS, CHUNKS, 128, HW], f32, kind="ExternalInput").ap()
    idx = nc.dram_tensor(
        "idx", [BS, CHUNKS, SH, 128, IDX_PER // 16], i16, kind="ExternalInput"
    ).ap()
    oc = nc.dram_tensor("oc", [BS, CHUNKS, 128, HW], f32, kind="ExternalOutput").ap()

    with tile.TileContext(nc) as tc:
        with (
            tc.tile_pool(name="big", bufs=4) as bigp,
            tc.tile_pool(name="small", bufs=3) as smp,
            tc.tile_pool(name="idxp", bufs=4) as idxp,
        ):
            for b in range(BS):
                for c in range(CHUNKS):
                    q_t = smp.tile([128, HW], f32, tag="q")
                    nc.sync.dma_start(out=q_t[:], in_=qc[b, c])
                    L = smp.tile([128, S * NH], f32, tag="L")
                    idx_ts = []
                    for sh in range(SH):
                        it = idxp.tile([128, IDX_PER // 16], i16, tag=f"idx{sh}")
                        nc.sync.dma_start(out=it[:], in_=idx[b, c, sh])
                        idx_ts.append(it)
                    for sh in range(SH):
                        kg = bigp.tile([128, SLOTS_PER, HW], f32, tag="g")
                        nc.gpsimd.dma_gather(
                            kg[:], kf, idx_ts[sh][:], IDX_PER, IDX_PER, HW,
                            queue_num=0,
                        )
                        # kg <- kg - q (broadcast q over the slot dim)
                        nc.vector.tensor_tensor(
                            out=kg[:],
                            in0=kg[:],
                            in1=q_t[:, None, :].to_broadcast([128, SLOTS_PER, HW]),
                            op=mybir.AluOpType.subtract,
                        )
                        # L[:, sh half] = sum_w |kg|   ([128, s*h])
                        nc.vector.tensor_reduce(
                            out=L[:, sh * SLOTS_PER * NH : (sh + 1) * SLOTS_PER * NH],
                            in_=kg[:].rearrange("p s (h w) -> p (s h) w", w=W),
                            axis=mybir.AxisListType.X,
                            op=mybir.AluOpType.add,
                            apply_absolute_value=True,
                        )
                    # --- softmax over s (strided views: L is [p, (s h)]) ---
                    Lv = L[:].rearrange("p (s h) -> p h s", h=NH)
                    Lmin = smp.tile([128, NH], f32, tag="lmin")
                    nc.vector.tensor_reduce(
                        out=Lmin[:], in_=Lv, axis=mybir.AxisListType.X,
                        op=mybir.AluOpType.min,
                    )
                    E = smp.tile([128, S * NH], f32, tag="E")
                    nc.vector.tensor_tensor(
                        out=E[:].rearrange("p (s h) -> p s h", h=NH),
                        in0=L[:].rearrange("p (s h) -> p s h", h=NH),
                        in1=Lmin[:, None, :].to_broadcast([128, S, NH]),
                        op=mybir.AluOpType.subtract,
                    )
                    nc.scalar.activation(
                        out=E[:], in_=E[:], func=mybir.ActivationFunctionType.Exp,
                        scale=-1.0 / np.sqrt(W),
                    )
                    den = smp.tile([128, NH], f32, tag="den")
                    nc.vector.tensor_reduce(
                        out=den[:],
                        in_=E[:].rearrange("p (s h) -> p h s", h=NH),
                        axis=mybir.AxisListType.X,
                        op=mybir.AluOpType.add,
                    )
                    rden = smp.tile([128, NH], f32, tag="rden")
                    nc.vector.reciprocal(rden[:], den[:])
                    Wt = smp.tile([128, S * NH], f32, tag="Wt")
                    nc.vector.tensor_tensor(
                        out=Wt[:].rearrange("p (s h) -> p s h", h=NH),
                        in0=E[:].rearrange("p (s h) -> p s h", h=NH),
                        in1=rden[:, None, :].to_broadcast([128, S, NH]),
                        op=mybir.AluOpType.mult,
                    )
                    # --- weighted v gather+sum ---
                    ot = None
                    for sh in range(SH):
                        vg = bigp.tile([128, SLOTS_PER, HW], f32, tag="g")
                        nc.gpsimd.dma_gather(
                            vg[:], vf, idx_ts[sh][:], IDX_PER, IDX_PER, HW,
                            queue_num=0,
                        )
                        wslice = Wt[:, sh * SLOTS_PER * NH : (sh + 1) * SLOTS_PER * NH]
                        nc.vector.tensor_tensor(
                            out=vg[:].rearrange("p s (h w) -> p s h w", w=W),
                            in0=vg[:].rearrange("p s (h w) -> p s h w", w=W),
                            in1=wslice.rearrange("p (s h) -> p s h", h=NH)[
                                :, :, :, None
                            ].to_broadcast([128, SLOTS_PER, NH, W]),
                            op=mybir.AluOpType.mult,
                        )
                        on = smp.tile([128, HW], f32, tag="on")
                        nc.vector.tensor_reduce(
                            out=on[:],
                            in_=vg[:].rearrange("p s hw -> p hw s"),
                            axis=mybir.AxisListType.X,
                            op=mybir.AluOpType.add,
                        )
                        if ot is None:
                            ot = on
                        else:
                            acc = smp.tile([128, HW], f32, tag="acc")
                            nc.vector.tensor_tensor(
                                out=acc[:], in0=ot[:], in1=on[:],
                                op=mybir.AluOpType.add,
                            )
                            ot = acc
                    nc.sync.dma_start(out=oc[b, c], in_=ot[:])
    nc.compile()
    return nc


_NC_CACHE = None
_LAST_IN_MAPS = None


def kernel(v, q, k, coo, dst_mxlen):
    global _NC_CACHE
    assert int(dst_mxlen) == S
    v = np.asarray(v, dtype=np.float32)
    q = np.asarray(q, dtype=np.float32)
    k = np.asarray(k, dtype=np.float32)
    coo = np.asarray(coo)

    # src table: srct[t, s] = src index of edge (dst=t, slot=s)
    srct = np.zeros((N_TOK, S), dtype=np.int64)
    srct[coo[:, 0], coo[:, 2]] = coo[:, 1]

    kf = k.reshape(BS * N_TOK, HW)
    vf = v.reshape(BS * N_TOK, HW)

    if _NC_CACHE is None:
        _NC_CACHE = build_kernel()
    nc = _NC_CACHE

    in_maps = []
    for core in range(N_CORES):
        lo = core * DT
        qc = q[:, lo : lo + DT].reshape(BS, CHUNKS, 128, HW)
        idx = np.zeros((BS, CHUNKS, SH, 128, IDX_PER // 16), dtype=np.int16)
        for b in range(BS):
            for c in range(CHUNKS):
                for sh in range(SH):
                    # index i = s_local*128 + p  ->  row b*2048 + srct[...]
                    sl = np.arange(SLOTS_PER) + sh * SLOTS_PER
                    flat = (
                        b * N_TOK
                        + srct[lo + c * 128 : lo + (c + 1) * 128, sl].T
                    ).reshape(-1).astype(np.int16)  # [s_local, p] -> flat
                    idx[b, c, sh] = _wrap_idx(flat)
        in_maps.append(
            {"kf": kf, "vf": vf, "qc": np.ascontiguousarray(qc), "idx": idx}
        )

    global _LAST_IN_MAPS
    _LAST_IN_MAPS = in_maps
    res = run_bass_kernel_spmd(nc, in_maps, list(range(N_CORES)))
    out = np.empty((BS, N_TOK, NH, W), dtype=np.float32)
    for core in range(N_CORES):
        lo = core * DT
        out[:, lo : lo + DT] = res.results[core]["oc"].reshape(BS, DT, NH, W)
    return out

